# revision 15
# baseline (speedup 1.0000x reference)
"""DiT block kernel for Trainium2, data-parallel over batch across 8 NeuronCores.

Per-core layout: all activations are kept transposed ([feature, token]) so that
every GEMM consumes weights in their native DRAM [in, out] layout as lhsT and
activations as rhs, with no on-device transposes anywhere.

  - adaLN modulation: silu(c) @ w_ada computed as a [1, 6144] row on PE, then
    bounced through DRAM to get per-partition column layout for the 6 vectors.
  - LayerNorm (transposed): stats via ones-matmul on PE (sum, sum of squares
    over the partition/feature dim), per-token scale a=rstd / offset b=-mu*rstd
    broadcast across partitions with a stride-0 DMA from a DRAM scratch row.
  - Attention per head: scores^T via K=64 matmuls (two heads share one
    128-partition tile at bases 0/64), exp on ScalarE straight out of PSUM
    (scale=1/8, no max subtraction: scores are O(1) here), o^T accumulated with
    an augmented [v | 1] lhsT so the softmax denominator falls out as row 64,
    then one reciprocal + stride-0 broadcast + multiply per head.
  - Residuals/biases: per-partition scalars fused into tensor_scalar /
    scalar_tensor_tensor ops; bias of the (token-partition) v GEMM is folded in
    as an extra K=1 matmul with a ones row.
  - SBUF pressure: x and the x1 residual stream live in DRAM and are streamed;
    the MLP runs per token-half so the hidden activation tile is [128,32,512].

Host side shards B=8 one element per core, pre-transposes x, pre-casts weights
to bf16, and transposes the per-core [D, N] outputs back.
"""
import sys

for _p in ("/opt/trn_rl_repo",):
    if _p not in sys.path:
        sys.path.insert(0, _p)

import numpy as np
import ml_dtypes
from contextlib import ExitStack

import concourse.bass as bass
import concourse.mybir as mybir
import concourse.tile as tile

f32 = mybir.dt.float32
bf16 = mybir.dt.bfloat16
AF = mybir.ActivationFunctionType
OP = mybir.AluOpType

P = 128
NTOK = 1024     # tokens per batch element
D = 1024        # model dim
KD = D // P     # 8 k-tiles over model dim
H = 16          # heads
DH = 64         # head dim
F = 4096        # mlp hidden
KF = F // P     # 32
ADA = 6 * D     # 6144
EPS = 1e-6
NCORES = 8


def _split_multi_waits(nc):
    """This container's walrus build encodes at most ONE sync wait per
    instruction ("Too many sync wait commands"); hoist extra waits onto
    single-wait NoOps in the same engine stream."""
    for fn in nc.m.functions:
        for blk in fn.blocks:
            out = []
            for inst in blk.instructions:
                si = inst.sync_info
                waits = list(si.on_wait) if si is not None and si.on_wait else []
                if len(waits) > 1:
                    for i, w in enumerate(waits[:-1]):
                        nop = mybir.InstNoOp(name=f"{inst.name}-ws{i}", ins=[], outs=[])
                        nop.engine = inst.engine
                        nop.sync_info = mybir.SyncInfo(on_wait=[w], on_update=[])
                        out.append(nop)
                    inst.sync_info = mybir.SyncInfo(on_wait=[waits[-1]],
                                                    on_update=list(si.on_update))
                out.append(inst)
            blk.instructions = out


def build_nc(sim_gelu=False, split_waits=True):
    nc = bass.Bass(trn_type="TRN2")

    xT_d = nc.dram_tensor("xT", [D, NTOK], f32, kind="ExternalInput")
    ccol_d = nc.dram_tensor("ccol", [P, KD], f32, kind="ExternalInput")
    wqkv_d = nc.dram_tensor("wqkv", [D, 3 * D], bf16, kind="ExternalInput")
    bqk_col_d = nc.dram_tensor("bqk_col", [P, 16], f32, kind="ExternalInput")
    bv_row_d = nc.dram_tensor("bv_row", [1, D], bf16, kind="ExternalInput")
    wproj_d = nc.dram_tensor("wproj", [D, D], bf16, kind="ExternalInput")
    bproj_col_d = nc.dram_tensor("bproj_col", [P, KD], f32, kind="ExternalInput")
    wmlp1_d = nc.dram_tensor("wmlp1", [D, F], bf16, kind="ExternalInput")
    bmlp1_col_d = nc.dram_tensor("bmlp1_col", [P, KF], f32, kind="ExternalInput")
    wmlp2_d = nc.dram_tensor("wmlp2", [F, D], bf16, kind="ExternalInput")
    bmlp2_col_d = nc.dram_tensor("bmlp2_col", [P, KD], f32, kind="ExternalInput")
    wada_d = nc.dram_tensor("wada", [D, ADA], bf16, kind="ExternalInput")
    bada_row_d = nc.dram_tensor("bada_row", [1, ADA], bf16, kind="ExternalInput")
    outT_d = nc.dram_tensor("outT", [D, NTOK], f32, kind="ExternalOutput")

    # [in, out] weights viewed as [p, ktile, out]
    wqkv_r = wqkv_d.rearrange("(k p) m -> p k m", p=P)
    wproj_r = wproj_d.rearrange("(k p) m -> p k m", p=P)
    wmlp1_r = wmlp1_d.rearrange("(k p) m -> p k m", p=P)
    wmlp2_r = wmlp2_d.rearrange("(k p) m -> p k m", p=P)
    wada_r = wada_d.rearrange("(k p) m -> p k m", p=P)
    xT_r = xT_d.rearrange("(mt p) t -> p mt t", p=P)
    outT_r = outT_d.rearrange("(mt p) t -> p mt t", p=P)

    def bcast_ap(dram_tile, nparts, nelem):
        return bass.AP(tensor=dram_tile.tensor, offset=dram_tile.offset,
                       ap=[[0, nparts], [1, nelem]])

    with tile.TileContext(nc) as tc, ExitStack() as ctx:
        persist = ctx.enter_context(tc.tile_pool(name="persist", bufs=1))
        w8 = ctx.enter_context(tc.tile_pool(name="w8", bufs=3))       # [P, KD, 128] chunks
        wada_p = ctx.enter_context(tc.tile_pool(name="wadap", bufs=2))
        tmp = ctx.enter_context(tc.tile_pool(name="tmp", bufs=2))     # [P, NTOK] work tiles
        rows = ctx.enter_context(tc.tile_pool(name="rows", bufs=1))   # [1, NTOK] stat rows
        ebuf = ctx.enter_context(tc.tile_pool(name="ebuf", bufs=2))
        ps_mm = ctx.enter_context(tc.tile_pool(name="psmm", bufs=2, space="PSUM"))
        ps_o = ctx.enter_context(tc.tile_pool(name="pso", bufs=2, space="PSUM"))
        dram = ctx.enter_context(tc.tile_pool(name="drsc", bufs=2, space="DRAM"))

        ones_f = persist.tile([P, 1], f32)
        nc.vector.memset(ones_f, 1.0)
        onesrow_b = persist.tile([1, P], bf16)
        nc.vector.memset(onesrow_b, 1.0)
        eps_t = persist.tile([1, 1], f32)
        nc.vector.memset(eps_t, EPS)

        # ---------------- adaLN modulation ----------------
        ccol_sb = persist.tile([P, KD], f32)
        nc.sync.dma_start(ccol_sb[:], ccol_d[:])
        csig = persist.tile([P, KD], f32)
        nc.scalar.activation(csig[:], ccol_sb[:], AF.Sigmoid)
        scol = persist.tile([P, KD], bf16)
        nc.vector.tensor_mul(scol[:], ccol_sb[:], csig[:])

        mod_d = dram.tile([1, ADA], f32, tag="modd")
        NCH = 256
        modcol1 = persist.tile([P, 16], f32)
        for chk in range(ADA // NCH):
            sl = slice(chk * NCH, (chk + 1) * NCH)
            wada_t = wada_p.tile([P, KD, NCH], bf16, tag="wada")
            nc.sync.dma_start(wada_t[:], wada_r[:, :, sl])
            ps = ps_mm.tile([P, 1024], f32, tag="mm")
            for kt in range(KD):
                nc.tensor.matmul(ps[0:1, 0:NCH], lhsT=scol[:, kt:kt + 1],
                                 rhs=wada_t[:, kt, :], start=(kt == 0), stop=False)
            bada_t = tmp.tile([1, NCH], bf16, tag="badach")
            nc.sync.dma_start(bada_t[:], bada_row_d[0:1, sl])
            nc.tensor.matmul(ps[0:1, 0:NCH], lhsT=onesrow_b[0:1, 0:1],
                             rhs=bada_t[0:1, :], start=False, stop=True)
            mr = rows.tile([1, NCH], f32, tag="modr")
            nc.scalar.activation(mr[0:1, :], ps[0:1, 0:NCH], AF.Copy)
            nc.sync.dma_start(mod_d[0:1, sl], mr[:])
            if chk * NCH + NCH == 2 * D:
                # g1 | be1 written -> fetch column layout early
                nc.sync.dma_start(
                    modcol1[:], mod_d[0:1, 0:2 * D].rearrange("o (j p) -> p (o j)", p=P))
        modcol2 = persist.tile([P, 32], f32)
        nc.sync.dma_start(
            modcol2[:], mod_d[0:1, 2 * D:6 * D].rearrange("o (j p) -> p (o j)", p=P))

        gp1 = persist.tile([P, KD], f32)
        nc.vector.tensor_scalar_add(gp1[:], modcol1[:, 0:8], 1.0)
        be1col = modcol1[:, 8:16]
        a1col = modcol2[:, 0:8]
        gp2 = persist.tile([P, KD], f32)
        nc.vector.tensor_scalar_add(gp2[:], modcol2[:, 8:16], 1.0)
        be2col = modcol2[:, 16:24]
        a2col = modcol2[:, 24:32]

        # ---------------- transposed layernorm ----------------
        def layernorm_mod(src_r, gpcol, becol, out_bf, abtag):
            """src_r: DRAM view [p, kt, tok]. out_bf[:, kt, :] (bf16) =
            ((x - mu) * rstd) * gpcol + becol, stats over the feature dim."""
            ps_sum = ps_mm.tile([P, 1024], f32, tag="mm")
            ps_sq = ps_mm.tile([P, 1024], f32, tag="mm")
            for kt in range(KD):
                xt = tmp.tile([P, NTOK], f32, tag="xstream")
                nc.sync.dma_start(xt[:], src_r[:, kt, :])
                xsq = tmp.tile([P, NTOK], f32, tag="lnwork")
                nc.scalar.activation(xsq[:], xt[:], AF.Square)
                for h0 in (0, 512):
                    nc.tensor.matmul(ps_sum[0:1, h0:h0 + 512], lhsT=ones_f[:, 0:1],
                                     rhs=xt[:, h0:h0 + 512],
                                     start=(kt == 0), stop=(kt == KD - 1))
                    nc.tensor.matmul(ps_sq[0:1, h0:h0 + 512], lhsT=ones_f[:, 0:1],
                                     rhs=xsq[:, h0:h0 + 512],
                                     start=(kt == 0), stop=(kt == KD - 1))
            murow = rows.tile([1, NTOK], f32, tag="murow")
            nc.scalar.activation(murow[0:1, :], ps_sum[0:1, :], AF.Copy, scale=1.0 / D)
            msqrow = rows.tile([1, NTOK], f32, tag="msqrow")
            nc.scalar.activation(msqrow[0:1, :], ps_sq[0:1, :], AF.Copy, scale=1.0 / D)
            srow = rows.tile([1, NTOK], f32, tag="srow")
            nc.vector.tensor_mul(srow[0:1, :], murow[0:1, :], murow[0:1, :])
            nc.vector.tensor_sub(msqrow[0:1, :], msqrow[0:1, :], srow[0:1, :])
            nc.scalar.activation(srow[0:1, :], msqrow[0:1, :], AF.Sqrt, bias=eps_t[0:1, :])
            ab = rows.tile([1, 2 * NTOK], f32, tag="abrow")
            nc.vector.reciprocal(ab[0:1, 0:NTOK], srow[0:1, :])
            nc.vector.scalar_tensor_tensor(ab[0:1, NTOK:], murow[0:1, :], -1.0,
                                           ab[0:1, 0:NTOK], op0=OP.mult, op1=OP.mult)
            ab_d = dram.tile([1, 2 * NTOK], f32, tag="abd" + abtag)
            nc.sync.dma_start(ab_d[:], ab[:])
            abbc = tmp.tile([P, 2 * NTOK], f32, tag="abbc", bufs=1)
            nc.sync.dma_start(abbc[:], bcast_ap(ab_d, P, 2 * NTOK))
            for kt in range(KD):
                xt = tmp.tile([P, NTOK], f32, tag="xstream")
                nc.sync.dma_start(xt[:], src_r[:, kt, :])
                t1 = tmp.tile([P, NTOK], f32, tag="lnwork")
                nc.vector.tensor_mul(t1[:], xt[:], abbc[:, 0:NTOK])
                nc.vector.tensor_add(t1[:], t1[:], abbc[:, NTOK:])
                nc.vector.tensor_scalar(out=out_bf[:, kt, :], in0=t1[:],
                                        scalar1=gpcol[:, kt:kt + 1],
                                        scalar2=becol[:, kt:kt + 1],
                                        op0=OP.mult, op1=OP.add)

        # lifetime-scoped pools: A = {hT, wv} (LN1..vGEMM), B = {qkT, v}
        # (qkT GEMM..heads), C = {o} (heads..proj)
        cmC = tc.tile_pool(name="attnC", bufs=1)
        cmB = tc.tile_pool(name="attnB", bufs=1)
        cmA = tc.tile_pool(name="attnA", bufs=1)
        attnC, attnB, attnA = cmC.__enter__(), cmB.__enter__(), cmA.__enter__()
        if True:
            # ---------------- LN1 -> h ----------------
            hT = attnA.tile([P, KD, NTOK], bf16, tag="hT")
            layernorm_mod(xT_r, gp1, be1col, hT, "1")

            # ---------------- qk^T GEMM: [2048 out, 1024 tok] ----------------
            bqk_sb = persist.tile([P, 16], f32)
            nc.sync.dma_start(bqk_sb[:], bqk_col_d[:])
            qkT = attnB.tile([P, 16, NTOK], bf16, tag="qkT")
            for mt in range(16):
                wt = w8.tile([P, KD, P], bf16, tag="w8")
                nc.sync.dma_start(wt[:], wqkv_r[:, :, mt * P:(mt + 1) * P])
                ps = ps_mm.tile([P, 1024], f32, tag="mm")
                for h0 in (0, 512):
                    for kt in range(KD):
                        nc.tensor.matmul(ps[:, h0:h0 + 512], lhsT=wt[:, kt, :],
                                         rhs=hT[:, kt, h0:h0 + 512],
                                         start=(kt == 0), stop=(kt == KD - 1))
                nc.scalar.add(qkT[:, mt, :], ps[:, :], bqk_sb[:, mt:mt + 1])

            # ---------------- v GEMM: [1024 tok, 1024 vdims] ----------------
            bv_sb = persist.tile([1, D], bf16)
            nc.sync.dma_start(bv_sb[:], bv_row_d[:])
            wv_sb = attnA.tile([P, KD, D], bf16, tag="wv")
            nc.sync.dma_start(wv_sb[:], wqkv_r[:, :, 2 * D:3 * D])
            v_sb = attnB.tile([P, KD, H, DH + 1], bf16, tag="v")
            nc.vector.memset(v_sb[:, :, :, DH:DH + 1], 1.0)
            for mt in range(KD):  # token tiles
                ps = ps_mm.tile([P, 1024], f32, tag="mm")
                for h0 in (0, 512):
                    for kt in range(KD):
                        nc.tensor.matmul(ps[:, h0:h0 + 512],
                                         lhsT=hT[:, kt, mt * P:(mt + 1) * P],
                                         rhs=wv_sb[:, kt, h0:h0 + 512],
                                         start=(kt == 0), stop=False)
                    nc.tensor.matmul(ps[:, h0:h0 + 512], lhsT=onesrow_b[0:1, :],
                                     rhs=bv_sb[0:1, h0:h0 + 512], start=False, stop=True)
                nc.vector.tensor_copy(
                    out=v_sb[:, mt, :, 0:DH],
                    in_=ps.rearrange("p (h d) -> p h d", h=H))

            cmA.__exit__(None, None, None)  # free hT, wv

            # ---------------- attention, head by head ----------------
            o_sb = attnC.tile([P, KD, NTOK], bf16, tag="o")
            for h in range(H):
                pbase = (h % 2) * DH
                qtile, ktile = h // 2, 8 + h // 2
                oacc = ps_o.tile([P, 1024], f32, tag="oacc")
                for kt in range(KD):
                    sc = ps_mm.tile([P, 1024], f32, tag="mm")
                    lhsT_k = qkT[pbase:pbase + DH, ktile, kt * P:(kt + 1) * P]
                    for h0 in (0, 512):
                        nc.tensor.matmul(sc[:, h0:h0 + 512], lhsT=lhsT_k,
                                         rhs=qkT[pbase:pbase + DH, qtile, h0:h0 + 512],
                                         start=True, stop=True)
                    et = ebuf.tile([P, NTOK], bf16, tag="e")
                    nc.scalar.activation(et[:], sc[:, :], AF.Exp, scale=DH ** -0.5)
                    for h0 in (0, 512):
                        nc.tensor.matmul(oacc[0:DH + 1, h0:h0 + 512],
                                         lhsT=v_sb[:, kt, h, :],
                                         rhs=et[:, h0:h0 + 512],
                                         start=(kt == 0), stop=(kt == KD - 1))
                zrow = rows.tile([1, NTOK], f32, tag="zrow")
                nc.vector.reciprocal(zrow[0:1, :], oacc[DH:DH + 1, :])
                z_d = dram.tile([1, NTOK], f32, tag="zd")
                nc.sync.dma_start(z_d[:], zrow[:])
                zbc = tmp.tile([DH, NTOK], f32, tag="zbc", bufs=1)
                nc.sync.dma_start(zbc[:], bcast_ap(z_d, DH, NTOK))
                nc.vector.tensor_mul(o_sb[pbase:pbase + DH, h // 2, :],
                                     oacc[0:DH, :], zbc[:])

            cmB.__exit__(None, None, None)  # free qkT, v

            # ---------------- proj + residual -> x1 (DRAM) ----------------
            bproj_sb = persist.tile([P, KD], f32)
            nc.sync.dma_start(bproj_sb[:], bproj_col_d[:])
            x1_d = dram.tile([KD, P, NTOK], f32, tag="x1d")
            for mt in range(KD):
                wt = w8.tile([P, KD, P], bf16, tag="w8")
                nc.sync.dma_start(wt[:], wproj_r[:, :, mt * P:(mt + 1) * P])
                ps = ps_mm.tile([P, 1024], f32, tag="mm")
                for h0 in (0, 512):
                    for kt in range(KD):
                        nc.tensor.matmul(ps[:, h0:h0 + 512], lhsT=wt[:, kt, :],
                                         rhs=o_sb[:, kt, h0:h0 + 512],
                                         start=(kt == 0), stop=(kt == KD - 1))
                xmt = tmp.tile([P, NTOK], f32, tag="xstream")
                nc.sync.dma_start(xmt[:], xT_r[:, mt, :])
                tp = tmp.tile([P, NTOK], f32, tag="tproj")
                nc.vector.tensor_scalar(out=tp[:], in0=ps[:, :],
                                        scalar1=bproj_sb[:, mt:mt + 1],
                                        scalar2=a1col[:, mt:mt + 1],
                                        op0=OP.add, op1=OP.mult)
                nc.vector.tensor_add(tp[:], tp[:], xmt[:])
                nc.sync.dma_start(x1_d[mt], tp[:])
            cmC.__exit__(None, None, None)  # free o

        with tc.tile_pool(name="mlp", bufs=1) as mlp, \
             tc.tile_pool(name="w32", bufs=2) as w32:
            # ---------------- LN2 -> h2 ----------------
            x1_r = x1_d.rearrange("mt p t -> p mt t")
            h2T = mlp.tile([P, KD, NTOK], bf16, tag="h2T")
            layernorm_mod(x1_r, gp2, be2col, h2T, "2")

            # ---------------- mlp per token half ----------------
            bm1_sb = persist.tile([P, KF], f32)
            nc.sync.dma_start(bm1_sb[:], bmlp1_col_d[:])
            bm2_sb = persist.tile([P, KD], f32)
            nc.sync.dma_start(bm2_sb[:], bmlp2_col_d[:])
            for half in range(2):
                t0 = half * 512
                m1 = mlp.tile([P, KF, 512], bf16, tag="m1")
                for mt in range(KF):
                    wt = w8.tile([P, KD, P], bf16, tag="w8")
                    nc.sync.dma_start(wt[:], wmlp1_r[:, :, mt * P:(mt + 1) * P])
                    ps = ps_mm.tile([P, 1024], f32, tag="mm")
                    for kt in range(KD):
                        nc.tensor.matmul(ps[:, 0:512], lhsT=wt[:, kt, :],
                                         rhs=h2T[:, kt, t0:t0 + 512],
                                         start=(kt == 0), stop=(kt == KD - 1))
                    if sim_gelu:
                        # sim-only: x*sigmoid(1.702x) (CoreSim has no Gelu LUT)
                        yb = tmp.tile([P, 512], f32, tag="gy")
                        nc.scalar.add(yb[:, 0:512], ps[:, 0:512], bm1_sb[:, mt:mt + 1])
                        sg = tmp.tile([P, 512], f32, tag="gs")
                        nc.scalar.activation(sg[:, 0:512], yb[:, 0:512], AF.Sigmoid,
                                             scale=1.702)
                        nc.vector.tensor_mul(m1[:, mt, :], yb[:, 0:512], sg[:, 0:512])
                    else:
                        nc.scalar.activation(m1[:, mt, :], ps[:, 0:512], AF.Gelu,
                                             bias=bm1_sb[:, mt:mt + 1])
                for mt in range(KD):
                    wt = w32.tile([P, KF, P], bf16, tag="w32")
                    nc.sync.dma_start(wt[:], wmlp2_r[:, :, mt * P:(mt + 1) * P])
                    ps = ps_mm.tile([P, 1024], f32, tag="mm")
                    for kt in range(KF):
                        nc.tensor.matmul(ps[:, 0:512], lhsT=wt[:, kt, :],
                                         rhs=m1[:, kt, :],
                                         start=(kt == 0), stop=(kt == KF - 1))
                    xmt = tmp.tile([P, 512], f32, tag="xstream")
                    nc.sync.dma_start(xmt[:, 0:512], x1_r[:, mt, t0:t0 + 512])
                    tp = tmp.tile([P, 512], f32, tag="tproj")
                    nc.vector.tensor_scalar(out=tp[:, 0:512], in0=ps[:, 0:512],
                                            scalar1=bm2_sb[:, mt:mt + 1],
                                            scalar2=a2col[:, mt:mt + 1],
                                            op0=OP.add, op1=OP.mult)
                    nc.vector.tensor_add(tp[:, 0:512], tp[:, 0:512], xmt[:, 0:512])
                    nc.sync.dma_start(outT_r[:, mt, t0:t0 + 512], tp[:, 0:512])

    if split_waits:
        _split_multi_waits(nc)
    nc.finalize()
    return nc


def make_in_maps(x, c, w_qkv, b_qkv, w_proj, b_proj, w_mlp1, b_mlp1,
                 w_mlp2, b_mlp2, w_ada, b_ada):
    bf = ml_dtypes.bfloat16
    shared = {
        "wqkv": np.ascontiguousarray(np.asarray(w_qkv).astype(bf)),
        "bqk_col": np.ascontiguousarray(np.asarray(b_qkv)[:2 * D].astype(np.float32).reshape(16, P).T),
        "bv_row": np.ascontiguousarray(np.asarray(b_qkv)[2 * D:].astype(bf).reshape(1, D)),
        "wproj": np.ascontiguousarray(np.asarray(w_proj).astype(bf)),
        "bproj_col": np.ascontiguousarray(np.asarray(b_proj).astype(np.float32).reshape(KD, P).T),
        "wmlp1": np.ascontiguousarray(np.asarray(w_mlp1).astype(bf)),
        "bmlp1_col": np.ascontiguousarray(np.asarray(b_mlp1).astype(np.float32).reshape(KF, P).T),
        "wmlp2": np.ascontiguousarray(np.asarray(w_mlp2).astype(bf)),
        "bmlp2_col": np.ascontiguousarray(np.asarray(b_mlp2).astype(np.float32).reshape(KD, P).T),
        "wada": np.ascontiguousarray(np.asarray(w_ada).astype(bf)),
        "bada_row": np.ascontiguousarray(np.asarray(b_ada).astype(bf).reshape(1, ADA)),
    }
    in_maps = []
    for b in range(NCORES):
        m = dict(shared)
        m["xT"] = np.ascontiguousarray(np.asarray(x[b], dtype=np.float32).T)
        m["ccol"] = np.ascontiguousarray(np.asarray(c[b], dtype=np.float32).reshape(KD, P).T)
        in_maps.append(m)
    return in_maps


_NC_CACHE = None


def kernel(x, c, w_qkv, b_qkv, w_proj, b_proj, w_mlp1, b_mlp1,
           w_mlp2, b_mlp2, w_ada, b_ada, _trace=False, **_trace_kw):
    global _NC_CACHE
    from concourse.bass_utils import run_bass_kernel_spmd

    x = np.asarray(x)
    if _NC_CACHE is None:
        _NC_CACHE = build_nc()
    nc = _NC_CACHE
    in_maps = make_in_maps(x, c, w_qkv, b_qkv, w_proj, b_proj, w_mlp1, b_mlp1,
                           w_mlp2, b_mlp2, w_ada, b_ada)
    res = run_bass_kernel_spmd(nc, in_maps, core_ids=list(range(NCORES)),
                               trace=_trace, **_trace_kw)
    out = np.stack([res.results[b]["outT"].T for b in range(NCORES)])
    kernel.last_results = res
    return out.astype(np.float32)
